# revision 32
# baseline (speedup 1.0000x reference)
"""BernsteinConv Trainium2 Bass kernel (self-contained).

Strategy: dst-sharded across 8 NeuronCores (12500 nodes/core).
Per-core one-hot segment-sum on the PE via dma_gather of a
device-computed bf16 xs table (4 int16-indexed src ranges, 4 SWDGE
queues). Messages are single bf16 (tolerance 2e-2); aggregation is
PSUM-resident; dinv is host-precomputed from the edge list.
"""
import sys, types
import numpy as np


def _install_hooks():
    try:
        import antenv
    except Exception:
        return
    if "antenv.axon_hooks" in sys.modules:
        return
    hooks_mod = types.ModuleType("antenv.axon_hooks")
    _hook = [None]
    hooks_mod.set_axon_ntff_profile_hook = lambda h: _hook.__setitem__(0, h)
    hooks_mod.get_axon_ntff_profile_hook = lambda: _hook[0]
    sys.modules["antenv.axon_hooks"] = hooks_mod
    antenv.axon_hooks = hooks_mod
    try:
        from trn_agent_boot.trn_boot import _ntff_profile_via_ctypes
        hooks_mod.set_axon_ntff_profile_hook(
            _ntff_profile_via_ctypes("/opt/axon/libaxon_pjrt.so"))
    except Exception:
        pass
    import concourse.bass_utils as bass_utils
    bass_utils.upload_artifacts = lambda tmpdir: tmpdir


_install_hooks()

import concourse.bacc as bacc            # noqa: E402
import concourse.mybir as mybir          # noqa: E402
import concourse.tile as tile            # noqa: E402
import concourse.bass as bass            # noqa: E402
from concourse.bass_utils import run_bass_kernel_spmd  # noqa: E402


D = 32
WIN = 128
NC = 8
BW = 4          # blocks (of 128 edge slots) per (range, window) cell
BG = 8          # blocks per gather group (1024 idx -> 64+1 descs/engine)
NQ = 4


def _wrap16(flat):
    """dma_gather/scatter idx layout: flat pos s -> [s%16, s//16], x8 groups."""
    C = len(flat) // 16
    w16 = flat.reshape(C, 16).T
    col = np.zeros((128, C), dtype=np.int16)
    for grp in range(8):
        col[grp * 16:(grp + 1) * 16] = w16
    return col


def preprocess(feat, edge_src, edge_dst):
    """Host-side index preprocessing. Returns per-core input dicts + meta."""
    N = feat.shape[0]
    NPC = (N + NC - 1) // NC
    NPC_PAD = ((NPC + WIN - 1) // WIN) * WIN
    NWIN = NPC_PAD // WIN
    VPAD = ((N + 1023) // 1024) * 1024
    RSZ = VPAD // 4
    assert RSZ % 256 == 0 and RSZ <= 32768

    deg = np.bincount(edge_dst, minlength=N)
    dinv = (1.0 / np.sqrt(np.maximum(deg, 1).astype(np.float64))).astype(
        np.float32)

    src = edge_src.astype(np.int64)
    dst = edge_dst.astype(np.int64)
    core = dst // NPC
    rng_of = src // RSZ

    NBLK_R = NWIN * BW
    NBLK = 4 * NBLK_R
    CAP = BW * 128

    # first pass: per-core per-range cell assignment + overflow counts
    percore = []
    max_ovf_blocks = 1
    total_ovf = 0
    for c in range(NC):
        m = core == c
        s_c = src[m]
        d_loc = dst[m] - c * NPC
        w_c = d_loc // WIN
        off_c = d_loc % WIN
        r_c = rng_of[m]
        s_loc = s_c - r_c * RSZ

        ranges = []
        for r in range(4):
            mr = r_c == r
            wr = w_c[mr]
            order = np.argsort(wr, kind="stable")
            wr_s = wr[order]
            sl_s = s_loc[mr][order].astype(np.int16)
            of_s = off_c[mr][order].astype(np.int16)
            dl_s = d_loc[mr][order].astype(np.int16)
            cnts = np.bincount(wr_s, minlength=NWIN)
            cell_start = np.concatenate([[0], np.cumsum(cnts)[:-1]])
            pos = np.arange(len(wr_s)) - cell_start[wr_s]
            in_cell = pos < CAP
            ovf_n = int((~in_cell).sum())
            total_ovf += ovf_n
            max_ovf_blocks = max(max_ovf_blocks, (ovf_n + 127) // 128)
            ranges.append((wr_s, sl_s, of_s, dl_s, pos, in_cell))
        percore.append(ranges)

    OB = max_ovf_blocks  # overflow blocks per range (uniform across cores)

    GPR = (NBLK_R + BG - 1) // BG
    groups = []
    for r in range(4):
        for g in range(GPR):
            b0 = g * BG
            nb = min(BG, NBLK_R - b0)
            groups.append((r, b0, nb))

    # dinv chunk layout [128, 16*CV]: dinv_ch[p, k*CV+v] = dinv[k*RCH+p*CV+v]
    RCH = VPAD // 16
    CV = RCH // 128
    dinv_pad = np.zeros(VPAD, dtype=np.float32)
    dinv_pad[:N] = dinv
    dinv_ch = dinv_pad.reshape(16, 128, CV).transpose(1, 0, 2).reshape(
        128, 16 * CV)

    feat_pad = np.zeros((VPAD, D), dtype=np.float32)
    feat_pad[:N] = feat

    # overflow goes through per-range staging buffers: each ovf edge gets a
    # unique stage slot (window-major, one block of 128 per window per range),
    # then each stage re-enters the one-hot matmul path as one extra block
    # per window per range.
    STG = NWIN * 128                # stage rows per range (plus a dump block)
    OVB = 1
    assert STG + 128 <= 32768

    in_maps = []
    for c in range(NC):
        srcs = np.zeros((NBLK, 128), dtype=np.int16)
        dstoff = np.full((NBLK, 128), 999, dtype=np.int16)
        ovf_src = np.zeros((4, OB * 128), dtype=np.int16)
        ovf_stg = np.full((4, OB * 128), STG, dtype=np.int16)  # dump row
        ovf_doff = np.full((4, NWIN, 128), 999, dtype=np.int16)
        for r in range(4):
            wr_s, sl_s, of_s, dl_s, pos, in_cell = percore[c][r]
            blk = (r * NWIN + wr_s[in_cell]) * BW + pos[in_cell] // 128
            slot = pos[in_cell] % 128
            srcs[blk, slot] = sl_s[in_cell]
            dstoff[blk, slot] = of_s[in_cell]
            novf = int((~in_cell).sum())
            ow = wr_s[~in_cell]          # sorted by window
            oof = of_s[~in_cell]
            iw = np.arange(novf) - np.searchsorted(ow, ow)
            assert iw.max(initial=0) < 128
            stg = ow * 128 + iw
            ovf_src[r, :novf] = sl_s[~in_cell]
            ovf_stg[r, :novf] = stg.astype(np.int16)
            ovf_doff[r, ow, iw] = oof

        idx_cols = []
        for (r, b0, nb) in groups:
            blkix = r * NBLK_R + b0 + np.arange(nb)
            idx_cols.append(_wrap16(srcs[blkix].reshape(-1)))
        idxG = np.concatenate(idx_cols, axis=1)
        doffG = dstoff.T.copy()  # [128, NBLK], range-major block order
        ovfsrc = np.concatenate([_wrap16(ovf_src[r]) for r in range(4)],
                                axis=1)
        ovfdst = np.concatenate([_wrap16(ovf_stg[r]) for r in range(4)],
                                axis=1)
        ovfdoff = ovf_doff.transpose(2, 0, 1).reshape(128, 4 * NWIN).copy()

        lo, hi = c * NPC, min((c + 1) * NPC, N)
        nloc = hi - lo
        fl = np.zeros((NPC_PAD, D), dtype=np.float32)
        fl[:nloc] = feat[lo:hi]
        featl = fl.reshape(NWIN, 128, D).transpose(1, 0, 2).reshape(
            128, NWIN * D)
        dl = np.zeros(NPC_PAD, dtype=np.float32)
        dl[:nloc] = dinv[lo:hi]
        dinvl = dl.reshape(NWIN, 128).T.copy()

        in_maps.append({
            "featfull": feat_pad,
            "dinvch": dinv_ch,
            "idx": idxG,
            "doff": doffG,
            "ovfsrc": ovfsrc,
            "ovfdst": ovfdst,
            "ovfdoff": ovfdoff,
            "dinvl": dinvl,
            "featl": featl,
        })
    meta = dict(N=N, NPC=NPC, NPC_PAD=NPC_PAD, NWIN=NWIN, VPAD=VPAD, RSZ=RSZ,
                NBLK_R=NBLK_R, NBLK=NBLK, groups=groups, GPR=GPR,
                idx_cols=in_maps[0]["idx"].shape[1], OB=OB, OVB=OVB,
                STG=STG, has_ovf=bool(total_ovf > 0))
    return in_maps, meta


def build(nc, meta):
    """Emit the kernel program onto nc."""
    import contextlib
    dt = mybir.dt
    NWIN = meta["NWIN"]; VPAD = meta["VPAD"]; RSZ = meta["RSZ"]
    NBLK_R = meta["NBLK_R"]; NBLK = meta["NBLK"]
    GPR = meta["GPR"]; OB = meta["OB"]; OVB = meta["OVB"]
    STG = meta["STG"]
    NPC_PAD = meta["NPC_PAD"]; HAS_OVF = meta.get("has_ovf", True)
    RCH = VPAD // 16
    CV = RCH // 128

    t_feat = nc.dram_tensor("featfull", [VPAD, D], dt.float32,
                            kind="ExternalInput")
    t_dinvch = nc.dram_tensor("dinvch", [128, 16 * CV], dt.float32,
                              kind="ExternalInput")
    t_idx = nc.dram_tensor("idx", [128, meta["idx_cols"]], dt.int16,
                           kind="ExternalInput")
    t_doff = nc.dram_tensor("doff", [128, NBLK], dt.int16,
                            kind="ExternalInput")
    t_ovfs = nc.dram_tensor("ovfsrc", [128, 4 * OB * 8], dt.int16,
                            kind="ExternalInput")
    t_ovfd = nc.dram_tensor("ovfdst", [128, 4 * OB * 8], dt.int16,
                            kind="ExternalInput")
    t_dinvl = nc.dram_tensor("dinvl", [128, NWIN], dt.float32,
                             kind="ExternalInput")
    t_featl = nc.dram_tensor("featl", [128, NWIN * D], dt.float32,
                             kind="ExternalInput")
    t_out = nc.dram_tensor("outl", [128, NWIN * D], dt.float32,
                           kind="ExternalOutput")
    t_xs = [nc.dram_tensor(f"xs{r}", [RSZ, 128], dt.bfloat16, kind="Internal")
            for r in range(4)]
    t_stg = [nc.dram_tensor(f"ovfstage{r}", [STG + 128, 64], dt.float32,
                            kind="Internal") for r in range(4)]
    t_ovfdoff = nc.dram_tensor("ovfdoff", [128, 4 * NWIN], dt.int16,
                               kind="ExternalInput")

    gcol = []   # idx-slab column offset per group
    off = 0
    for (r, b0, nb) in meta["groups"]:
        gcol.append(off)
        off += nb * 8

    qctr = [0]

    def next_q():
        q = qctr[0] % NQ
        qctr[0] += 1
        return q

    with tile.TileContext(nc) as tc:
        ctx = contextlib.ExitStack()
        with ctx:
            consts = ctx.enter_context(tc.tile_pool(name="consts", bufs=1))
            xin = ctx.enter_context(tc.tile_pool(name="xin", bufs=2))
            xout = ctx.enter_context(tc.tile_pool(name="xout", bufs=2))
            big = ctx.enter_context(tc.tile_pool(name="big", bufs=1))
            msgp = ctx.enter_context(tc.tile_pool(name="msgp", bufs=14))
            sp = ctx.enter_context(tc.tile_pool(name="sp", bufs=4))
            slabp = ctx.enter_context(tc.tile_pool(name="slabp", bufs=1))
            psump = ctx.enter_context(tc.tile_pool(name="psum", bufs=1,
                                                   space="PSUM"))

            iotaB = consts.tile([128, BG, 128], dt.int16)
            nc.gpsimd.iota(iotaB[:], pattern=[[0, BG], [1, 128]], base=0,
                           channel_multiplier=0)
            iotaO = consts.tile([128, 16, 128], dt.int16)
            nc.gpsimd.iota(iotaO[:], pattern=[[0, 16], [1, 128]], base=0,
                           channel_multiplier=0)
            ovp = ctx.enter_context(tc.tile_pool(name="ovp", bufs=2))
            ovfp = ctx.enter_context(tc.tile_pool(name="ovfp", bufs=6))

            from concourse.tile import add_dep_helper
            RCOLS = NBLK_R * 8

            # dinv table first (needed by the very first chunk multiply)
            dinva = consts.tile([128, 16 * CV], dt.float32)
            nc.scalar.dma_start(dinva[:], t_dinvch[:])

            # ---- phase A: xs tables (bf16, scaled), 16 contiguous-row chunks
            # in on sync ring, out on scalar ring to pipeline. idx/doff slab
            # loads are interleaved per range so range 0 is ready ASAP.
            # Full 256B rows are written contiguously (payload in cols 0:D,
            # zeros elsewhere from the one-time memsets below).
            for b in range(2):
                xz = xout.tile([128, CV, 128], dt.bfloat16, tag="xch")
                nc.vector.memset(xz[:], 0.0)
            slabs = []
            for k in range(16):
                fin = xin.tile([128, CV, D], dt.float32, tag="fin")
                nc.sync.dma_start(
                    fin[:],
                    t_feat[k * RCH:(k + 1) * RCH, :].rearrange(
                        "(p v) d -> p v d", p=128))
                xch = xout.tile([128, CV, 128], dt.bfloat16, tag="xch")
                degf = dinva[:, k * CV:(k + 1) * CV]
                nc.vector.tensor_tensor(
                    out=xch[:, :, 0:D], in0=fin[:],
                    in1=degf.unsqueeze(2).broadcast_to([128, CV, D]),
                    op=mybir.AluOpType.mult)
                r = k // 4
                if k % 4 == 0:
                    slab_i = slabp.tile([128, RCOLS], dt.int16,
                                        tag=f"slab_i{r}", name=f"slab_i{r}")
                    nc.scalar.dma_start(slab_i[:],
                                        t_idx[:, r * RCOLS:(r + 1) * RCOLS])
                    slab_d = slabp.tile([128, NBLK_R], dt.int16,
                                        tag=f"slab_d{r}", name=f"slab_d{r}")
                    nc.scalar.dma_start(
                        slab_d[:], t_doff[:, r * NBLK_R:(r + 1) * NBLK_R])
                    slabs.append((slab_i, slab_d))
                lo_row = (k % 4) * RCH
                weng = nc.sync if k % 2 == 0 else nc.scalar
                weng.dma_start(
                    t_xs[r][lo_row:lo_row + RCH, :].rearrange(
                        "(p v) d -> p v d", p=128),
                    xch[:])

            # ---- late loads: overflow slabs + local feat/dinv (phase C/D)
            if HAS_OVF:
                ovf_i = slabp.tile([128, 4 * OB * 8], dt.int16, tag="ovf_i")
                nc.scalar.dma_start(ovf_i[:], t_ovfs[:])
                ovf_d = slabp.tile([128, 4 * OB * 8], dt.int16, tag="ovf_d")
                nc.scalar.dma_start(ovf_d[:], t_ovfd[:])
                ovf_do = slabp.tile([128, 4 * NWIN], dt.int16,
                                    tag="ovf_do")
                nc.scalar.dma_start(ovf_do[:], t_ovfdoff[:])
            dinvl = big.tile([128, NWIN], dt.float32)
            nc.scalar.dma_start(dinvl[:], t_dinvl[:])
            featl = big.tile([128, NWIN * D], dt.float32)
            nc.scalar.dma_start(featl[:], t_featl[:])

            # ---- zero the overflow stage (sync ring, after fins)
            zdeps = []
            if HAS_OVF:
                zrow = big.tile([128, 49 * 64], dt.float32)
                nc.vector.memset(zrow[:], 0.0)
                z3 = zrow[:].rearrange("p (w d) -> p w d", d=64)
                for r in range(4):
                    r0 = 0
                    while r0 < STG:
                        nrw = min(6272, STG - r0)
                        z = nc.sync.dma_start(
                            t_stg[r][r0:r0 + nrw, :].rearrange(
                                "(w p) d -> p w d", p=128),
                            z3[:, :nrw // 128, :])
                        zdeps.append(z)
                        r0 += nrw
                    z = nc.sync.dma_start(t_stg[r][STG:STG + 128, :],
                                          zrow[:, 0:64])
                    zdeps.append(z)

            # ---- phase C: gather + segsum, per range, window sweep
            scatters = []
            psb = []
            for i in range(7):
                psb_i = psump.tile([128, 512], dt.float32, tag=f"ps{i}",
                                   name=f"psb{i}")
                psb.append(psb_i)

            tiles = {}

            def do_gather(r, g):
                gi = r * GPR + g
                (_, b0, nb) = meta["groups"][gi]
                C = nb * 8
                lcol = gcol[gi] - gcol[r * GPR]
                slab_i, _ = slabs[r]
                msg = msgp.tile([128, BG, 128], dt.bfloat16, tag="msg")
                nc.gpsimd.dma_gather(
                    out_ap=msg[:, :nb, :],
                    in_ap=t_xs[r][:],
                    idxs_ap=slab_i[:, lcol:lcol + C],
                    num_idxs=nb * 128, num_idxs_reg=nb * 128,
                    elem_size=128, single_packet=True,
                    queue_num=next_q())
                tiles[(r, g)] = msg

            def build_s(r, g):
                gi = r * GPR + g
                (_, b0, nb) = meta["groups"][gi]
                _, slab_d = slabs[r]
                S = sp.tile([128, BG, 128], dt.bfloat16, tag="S")
                nc.vector.tensor_tensor(
                    out=S[:, :nb, :],
                    in0=slab_d[:, b0:b0 + nb].unsqueeze(2).broadcast_to(
                        [128, nb, 128]),
                    in1=iotaB[:, :nb, :],
                    op=mybir.AluOpType.is_equal)
                return S

            ovf_tiles = {}

            def do_ovf_gather(r):
                # gather this range's overflow xs rows (single-packet chunks)
                tl = []
                for ov0 in range(0, OB, BG):
                    nbo = min(BG, OB - ov0)
                    c0 = r * OB * 8 + ov0 * 8
                    movf = ovfp.tile([128, BG, 128], dt.bfloat16,
                                     tag="movf")
                    nc.gpsimd.dma_gather(
                        out_ap=movf[:, :nbo, :],
                        in_ap=t_xs[r][:],
                        idxs_ap=ovf_i[:, c0:c0 + nbo * 8],
                        num_idxs=nbo * 128, num_idxs_reg=nbo * 128,
                        elem_size=128, single_packet=True,
                        queue_num=next_q())
                    tl.append((ov0, nbo, c0, movf))
                ovf_tiles[r] = tl

            def do_ovf_scatter(r, chunk=None):
                # scatter the gathered rows to their unique stage slots;
                # emitted well after the gather so the drain wait is over
                tl = ovf_tiles[r] if chunk is not None else ovf_tiles.pop(r)
                tl = [tl[chunk]] if chunk is not None else tl
                for (ov0, nbo, c0, movf) in tl:
                    sc = nc.gpsimd.dma_scatter_add(
                        out_ap=t_stg[r][:],
                        in_ap=movf[:, :nbo, :].bitcast(dt.float32),
                        idxs_ap=ovf_d[:, c0:c0 + nbo * 8],
                        num_idxs=nbo * 128, num_idxs_reg=nbo * 128,
                        elem_size=64, single_packet=True,
                        queue_num=next_q())
                    for z in zdeps:
                        add_dep_helper(sc.ins, z.ins,
                                       reason="stage zero before sc")
                    scatters.append(sc)

            t1 = big.tile([128, NWIN * D], dt.float32)
            t2 = (zrow if HAS_OVF
                  else big.tile([128, NWIN * D], dt.float32))
            d_done = set()

            def do_phase_d(i):
                if i in d_done:
                    return
                d_done.add(i)
                w0 = i * 16
                nw = min(16, NWIN - w0)
                cw = nw * D
                sl = slice(w0 * D, w0 * D + cw)
                v3 = lambda ap: ap.rearrange("p (w d) -> p w d", d=D)
                agg = psb[i][:, 0:cw]
                nc.vector.tensor_tensor(
                    out=v3(t1[:, sl]), in0=v3(agg),
                    in1=dinvl[:, w0:w0 + nw].unsqueeze(2).broadcast_to(
                        [128, nw, D]),
                    op=mybir.AluOpType.mult)
                nc.vector.tensor_tensor(
                    out=t1[:, sl], in0=featl[:, sl], in1=t1[:, sl],
                    op=mybir.AluOpType.subtract)
                nc.vector.tensor_scalar_mul(t2[:, sl], t1[:, sl], 0.5)
                nc.vector.tensor_tensor(
                    out=t2[:, sl], in0=featl[:, sl], in1=t2[:, sl],
                    op=mybir.AluOpType.subtract)
                nc.vector.tensor_tensor(
                    out=t1[:, sl], in0=t1[:, sl], in1=t2[:, sl],
                    op=mybir.AluOpType.mult)
                nc.sync.dma_start(t_out[:, sl], t1[:, sl])

            def do_ovf_pass(i):
                # reload each range's stage rows for bank i's windows as one
                # extra block per window, closing the bank's psum group
                w0 = i * 16
                nw = min(16, NWIN - w0)
                ovms, Sos = [], []
                for r in range(4):
                    ovm = ovp.tile([128, 16, 128], dt.bfloat16,
                                   tag="ovm", bufs=4)
                    ld = nc.sync.dma_start(
                        ovm[:, :nw, :],
                        t_stg[r][w0 * 128:(w0 + nw) * 128, :]
                        .bitcast(dt.bfloat16)
                        .rearrange("(g p) d -> p g d", p=128))
                    for sc in scatters:
                        add_dep_helper(ld.ins, sc.ins,
                                       reason="scatter before stage load")
                    So = sp.tile([128, 16, 128], dt.bfloat16,
                                 tag="So", bufs=4)
                    nc.vector.tensor_tensor(
                        out=So[:, :nw, :],
                        in0=ovf_do[:, r * NWIN + w0:r * NWIN + w0 + nw]
                        .unsqueeze(2).broadcast_to([128, nw, 128]),
                        in1=iotaO[:, :nw, :],
                        op=mybir.AluOpType.is_equal)
                    ovms.append(ovm)
                    Sos.append(So)
                for w in range(w0, w0 + nw):
                    ps = psb[i]
                    col = (w % 16) * D
                    for r in range(4):
                        nc.tensor.matmul(
                            out=ps[:, col:col + D],
                            lhsT=Sos[r][:, w - w0, :],
                            rhs=ovms[r][:, w - w0, 0:D],
                            start=False,
                            stop=((w % 16 == 15 or w == NWIN - 1)
                                  and r == 3))

            PREFETCH = 11
            for r in range(4):
                gathered = -1
                S_cur = None
                s_g = -1
                for w in range(NWIN):
                    last_blk = w * BW + BW - 1
                    need_g = last_blk // BG
                    while gathered < min(need_g + PREFETCH, GPR - 1):
                        gathered += 1
                        do_gather(r, gathered)
                    if HAS_OVF:
                        nch = (OB + BG - 1) // BG
                        if w == 6 and r < 3:
                            do_ovf_gather(r)
                        elif w == 55 and r == 2:
                            do_ovf_gather(3)
                        elif r < 3 and w >= 50 and (w - 50) % 12 == 0 \
                                and (w - 50) // 12 < nch:
                            do_ovf_scatter(r, (w - 50) // 12)
                        elif r == 3 and w < 2 * nch and w % 2 == 0:
                            do_ovf_scatter(3, w // 2)
                    ps = psb[w // 16]
                    col = (w % 16) * D
                    for k in range(BW):
                        blk_in_r = w * BW + k
                        g = blk_in_r // BG
                        bl = blk_in_r - g * BG
                        if g != s_g:
                            S_cur = build_s(r, g)
                            s_g = g
                        msg = tiles[(r, g)]
                        # start marks the whole 2KB zero-region (bank) as
                        # pending-zero; each window's first touch then
                        # overwrites its own 128B slice, later touches
                        # accumulate. So: start only on the bank's very first
                        # matmul, stop on its very last.
                        nc.tensor.matmul(
                            out=ps[:, col:col + D], lhsT=S_cur[:, bl, :],
                            rhs=msg[:, bl, 0:D],
                            start=(w % 16 == 0 and r == 0 and k == 0),
                            stop=(not HAS_OVF
                                  and (w % 16 == 15 or w == NWIN - 1)
                                  and r == 3 and k == BW - 1))
                        if g > 0 and bl == 0:
                            tiles.pop((r, g - 1), None)
                    if HAS_OVF and r == 3 and w % 16 == 15:
                        # close finished banks while r3 is still sweeping
                        do_ovf_pass(w // 16)
                        do_phase_d(w // 16)

            if HAS_OVF:
                # remaining banks (the partial last bank)
                do_ovf_pass(6)
            for i in range(7):
                do_phase_d(i)
    return "outl"


def postprocess(results, N, NPC, NWIN):
    out = np.zeros((N, D), dtype=np.float32)
    NPC_PAD = NWIN * 128
    for c, r in enumerate(results):
        o = r["outl"].reshape(128, NWIN, D).transpose(1, 0, 2).reshape(
            NPC_PAD, D)
        lo, hi = c * NPC, min((c + 1) * NPC, N)
        out[lo:hi] = o[:hi - lo]
    return out


def reference_np(feat, edge_src, edge_dst):
    N = feat.shape[0]
    deg = np.bincount(edge_dst, minlength=N).astype(np.float32)
    dinv = np.clip(deg, 1.0, None) ** -0.5
    xs = feat * dinv[:, None]
    agg = np.zeros_like(feat)
    np.add.at(agg, edge_dst, xs[edge_src])
    y = feat - agg * dinv[:, None]
    return y * (feat - y / 2)


_cache = {}


def kernel(feat, edge_src, edge_dst):
    feat = np.asarray(feat, dtype=np.float32)
    edge_src = np.asarray(edge_src)
    edge_dst = np.asarray(edge_dst)
    N = feat.shape[0]
    NPC = (N + NC - 1) // NC

    in_maps, meta = preprocess(feat, edge_src, edge_dst)

    key = (N, meta["NBLK"], meta["idx_cols"], meta["OB"], meta["has_ovf"])
    if key not in _cache:
        nc = bacc.Bacc("TRN2", target_bir_lowering=False, debug=False,
                       num_devices=NC, num_swdge_queues=NQ)
        build(nc, meta)
        nc.compile()
        _cache[key] = nc
    nc = _cache[key]

    trace = bool(getattr(kernel, "trace", False))
    if not getattr(kernel, "_warmed", False):
        run_bass_kernel_spmd(nc, in_maps, core_ids=list(range(NC)),
                             trace=False)
        kernel._warmed = True
    res = run_bass_kernel_spmd(nc, in_maps, core_ids=list(range(NC)),
                               trace=trace)
    kernel.last_exec_time_ns = res.exec_time_ns
    return postprocess(res.results, N, NPC, meta["NWIN"])
